# revision 3
# baseline (speedup 1.0000x reference)
"""Series decomposition: depthwise moving-average (box filter, W=25, replicate
padding) + remainder, data-parallel over batch across 8 NeuronCores.

Per core: x shard [4, 512, 4096] viewed as [2048, 4096] rows. For each
[128, 4096] row-block, build a replicate-padded block XP[128, 13+L+12], then
compute the sliding-window sum with a single DVE scan using the recurrence

    s[i] = s[i-1] + xp[i+12] - xp[i-13]

(tensor_tensor_scan: state = (data0 + state) - data1; the scan state is fp32
regardless of operand dtype), scale by the filter weight (1/25) on the scalar
engine, and subtract from x for the remainder. This is O(1) work per element
instead of O(W), so the kernel is HBM-bound.

HBM-traffic optimizations (the kernel sits at the HBM roofline):
- All device I/O is float16 (half-ulp 2^-11 ~ 5e-4 relative, far inside the
  tolerance): halves traffic vs fp32. fp32<->f16 conversion is host-side.
- K=4 row-blocks are grouped per DMA: one 4 MB input DMA (SP HWDGE ring) and
  one 8 MB output DMA (Activation HWDGE ring) per group — 8 DMAs per sweep
  instead of 48, amortizing the ~2us per-DMA completion latency and hitting
  the large-transfer DMA efficiency plateau.
- trend/remainder are interleaved in one output tensor [rows, 2, L] so a
  single strided DMA writes both; the host de-interleaves.
- The scan writes its window sums into the remainder slot (later overwritten
  by the actual remainder), so no separate fp32 sum buffer is needed and
  K=4 double-buffered groups fit in SBUF.
"""

import numpy as np

import concourse.bacc as bacc
import concourse.bass as bass
import concourse.mybir as mybir
from concourse.bass_utils import run_bass_kernel_spmd
from concourse.tile import TileContext

B, C, L, W = 32, 512, 4096, 25
PAD = W // 2  # 12
NCORES = 8
ROWS = (B // NCORES) * C  # 2048 rows per core
P = 128
NTILES = ROWS // P  # 16
LPAD = PAD + 1  # 13 left-pad cols (extra col feeds the scan's subtract lag)
XCOLS = LPAD + L + PAD  # 4121
KBLK = 4  # row-blocks per DMA group

FP32 = mybir.dt.float32
F16 = mybir.dt.float16


def build_nc(scale: float, rows: int = ROWS, l: int = L, repeats: int = 1,
             kblk: int = KBLK, bufs: int = 2) -> bass.Bass:
    """repeats>1 re-runs the whole sweep inside one NEFF (timing harnesses
    use this to make device time dominate per-call dispatch overhead)."""
    ntiles = rows // P
    ngroups = ntiles // kblk
    xcols = LPAD + l + PAD
    nc = bacc.Bacc(trn_type="TRN2")
    x = nc.dram_tensor("x", [rows, l], F16, kind="ExternalInput")
    out = nc.dram_tensor("out", [rows, 2, l], F16, kind="ExternalOutput")

    with TileContext(nc) as tc:
        with tc.tile_pool(name="pool", bufs=bufs) as pool:
            for it in range(ngroups * repeats):
                g = it % ngroups
                rsl = slice(g * kblk * P, (g + 1) * kblk * P)
                xpg = pool.tile([P, kblk, xcols], F16, tag="xpg")
                nc.sync.dma_start(
                    out=xpg[:, :, LPAD : LPAD + l],
                    in_=x[rsl, :].rearrange("(k p) l -> p k l", p=P),
                )
                tr = pool.tile([P, kblk, 2, l], F16, tag="tr")
                init = pool.tile([P, kblk], FP32, tag="init")
                for j in range(kblk):
                    # replicate ('edge') padding on both sides
                    nc.vector.tensor_copy(
                        out=xpg[:, j, 0:LPAD],
                        in_=xpg[:, j, LPAD : LPAD + 1].to_broadcast((P, LPAD)),
                    )
                    nc.vector.tensor_copy(
                        out=xpg[:, j, LPAD + l : xcols],
                        in_=xpg[:, j, LPAD + l - 1 : LPAD + l].to_broadcast((P, PAD)),
                    )
                    # window sum at i=-1 plus the lagged element the first
                    # scan step subtracts: sum of xp cols [-13..11] = XP[0:25]
                    nc.vector.tensor_reduce(
                        out=init[:, j : j + 1],
                        in_=xpg[:, j, 0:W],
                        axis=mybir.AxisListType.X,
                        op=mybir.AluOpType.add,
                    )
                for j in range(kblk):
                    # f16 window sums, staged in the remainder slot
                    nc.vector.tensor_tensor_scan(
                        out=tr[:, j, 1, :],
                        data0=xpg[:, j, W:xcols],
                        data1=xpg[:, j, 0:l],
                        initial=init[:, j : j + 1],
                        op0=mybir.AluOpType.add,
                        op1=mybir.AluOpType.subtract,
                    )
                for j in range(kblk):
                    nc.scalar.mul(tr[:, j, 0, :], tr[:, j, 1, :], scale)
                for j in range(kblk):
                    nc.vector.tensor_sub(
                        out=tr[:, j, 1, :],
                        in0=xpg[:, j, LPAD : LPAD + l],
                        in1=tr[:, j, 0, :],
                    )
                nc.scalar.dma_start(
                    out=out[rsl].rearrange("(k p) two l -> p k two l", p=P),
                    in_=tr[:, :, :, :],
                )
    nc.finalize()
    return nc


def _probe_devices():
    """Touch every NeuronCore with a trivial computation. After a previous
    client exits with in-flight bass executions, the first bass exec from a
    fresh client can fail with NRT_EXEC_UNIT_UNRECOVERABLE; a plain jax
    computation resets the state."""
    try:
        import jax
        import jax.numpy as jnp

        for d in jax.devices():
            y = jax.device_put(np.ones((4, 4), np.float32), d)
            jnp.sum(y).block_until_ready()
    except Exception:
        pass


def kernel(x, weight):
    x = np.ascontiguousarray(np.asarray(x, dtype=np.float32).astype(np.float16))
    # frozen depthwise moving-average kernel: every tap is 1/W
    scale = float(np.asarray(weight).reshape(-1)[0])
    nc = build_nc(scale)
    shards = x.reshape(NCORES, ROWS, L)
    in_maps = [{"x": shards[c]} for c in range(NCORES)]
    _probe_devices()
    out = None
    for attempt in range(3):
        try:
            out = run_bass_kernel_spmd(nc, in_maps, core_ids=list(range(NCORES)))
            break
        except Exception:
            if attempt == 2:
                raise
            # a dirty previous client session can leave the device mesh
            # "unrecoverable"; a fresh PJRT client + probe clears it
            try:
                import jax

                jax.clear_backends()
            except Exception:
                pass
            _probe_devices()
    packed = np.stack(
        [np.asarray(out.results[c]["out"]) for c in range(NCORES)], axis=0
    )  # [NCORES, ROWS, 2, L] f16
    trend = packed[:, :, 0, :].astype(np.float32).reshape(B, C, L)
    remainder = packed[:, :, 1, :].astype(np.float32).reshape(B, C, L)
    return trend, remainder


# revision 5
# speedup vs baseline: 1.3246x; 1.3246x over previous
"""Series decomposition: depthwise moving-average (box filter, W=25, replicate
padding) + remainder, data-parallel over batch across 8 NeuronCores.

Per core: x shard [4, 512, 4096] viewed as [2048, 4096] rows. For each
[128, 4096] row-tile, build a replicate-padded tile XP[128, 13+L+12], then
compute the sliding-window sum with a single DVE scan using the recurrence

    s[i] = s[i-1] + xp[i+12] - xp[i-13]

(tensor_tensor_scan: state = (data0 + state) - data1; the scan state is fp32
regardless of operand dtype), scale by the filter weight (1/25) on the scalar
engine, and subtract from x for the remainder. This is O(1) work per element
instead of O(W), so the kernel is HBM-bound.

HBM/DMA optimizations (the kernel sits at the HBM roofline):
- All device I/O is float16 (half-ulp 2^-11 ~ 5e-4 relative, far inside the
  2e-2 tolerance): halves HBM traffic vs fp32. fp32<->f16 conversion is
  host-side.
- The three DMA streams are spread across all three DMA-capable engines so
  their rings run concurrently: input on the SP HWDGE ring, trend on the
  Activation HWDGE ring, remainder on the GpSimd SWDGE path. A single ring
  serializes the per-DMA completion latency and measures ~190us/sweep; the
  3-ring spread measures ~145us, within ~5% of a pure-DMA floor probe.
- The scan stages its f16 window sums in the remainder tile (overwritten by
  the actual remainder afterwards), so no fp32 sum buffer is needed.
"""

import numpy as np

import concourse.bacc as bacc
import concourse.bass as bass
import concourse.mybir as mybir
from concourse.bass_utils import run_bass_kernel_spmd
from concourse.tile import TileContext

B, C, L, W = 32, 512, 4096, 25
PAD = W // 2  # 12
NCORES = 8
ROWS = (B // NCORES) * C  # 2048 rows per core
P = 128
NTILES = ROWS // P  # 16
LPAD = PAD + 1  # 13 left-pad cols (extra col feeds the scan's subtract lag)
XCOLS = LPAD + L + PAD  # 4121
BUFS = 6

FP32 = mybir.dt.float32
F16 = mybir.dt.float16


def build_nc(scale: float, rows: int = ROWS, l: int = L, repeats: int = 1,
             bufs: int = BUFS) -> bass.Bass:
    """repeats>1 re-runs the whole sweep inside one NEFF (timing harnesses
    use this to make device time dominate per-call dispatch overhead)."""
    ntiles = rows // P
    xcols = LPAD + l + PAD
    nc = bacc.Bacc(trn_type="TRN2")
    x = nc.dram_tensor("x", [rows, l], F16, kind="ExternalInput")
    trend = nc.dram_tensor("trend", [rows, l], F16, kind="ExternalOutput")
    remainder = nc.dram_tensor("remainder", [rows, l], F16, kind="ExternalOutput")

    with TileContext(nc) as tc:
        with tc.tile_pool(name="pool", bufs=bufs) as pool:
            for i in range(ntiles * repeats):
                i = i % ntiles
                rsl = slice(i * P, (i + 1) * P)
                xp = pool.tile([P, xcols], F16, tag="xp")
                nc.sync.dma_start(out=xp[:, LPAD : LPAD + l], in_=x[rsl, :])
                # replicate ('edge') padding on both sides
                nc.vector.tensor_copy(
                    out=xp[:, 0:LPAD],
                    in_=xp[:, LPAD : LPAD + 1].to_broadcast((P, LPAD)),
                )
                nc.vector.tensor_copy(
                    out=xp[:, LPAD + l : xcols],
                    in_=xp[:, LPAD + l - 1 : LPAD + l].to_broadcast((P, PAD)),
                )
                # window sum at i=-1 plus the lagged element the first scan
                # step subtracts: sum of xp cols [-13..11] = XP[:, 0:25]
                init = pool.tile([P, 1], FP32, tag="init")
                nc.vector.tensor_reduce(
                    out=init[:, 0:1],
                    in_=xp[:, 0:W],
                    axis=mybir.AxisListType.X,
                    op=mybir.AluOpType.add,
                )
                # f16 window sums, staged in the remainder tile
                r = pool.tile([P, l], F16, tag="r")
                nc.vector.tensor_tensor_scan(
                    out=r[:, :],
                    data0=xp[:, W:xcols],
                    data1=xp[:, 0:l],
                    initial=init[:, 0:1],
                    op0=mybir.AluOpType.add,
                    op1=mybir.AluOpType.subtract,
                )
                t = pool.tile([P, l], F16, tag="t")
                nc.scalar.mul(t[:, :], r[:, :], scale)
                nc.vector.tensor_sub(out=r[:, :], in0=xp[:, LPAD : LPAD + l], in1=t[:, :])
                nc.scalar.dma_start(out=trend[rsl, :], in_=t[:, :])
                nc.gpsimd.dma_start(out=remainder[rsl, :], in_=r[:, :])
    nc.finalize()
    return nc


def _probe_devices():
    """Touch every NeuronCore with a trivial computation. After a previous
    client exits with in-flight bass executions, the first bass exec from a
    fresh client can fail with NRT_EXEC_UNIT_UNRECOVERABLE; a plain jax
    computation resets the state."""
    try:
        import jax
        import jax.numpy as jnp

        for d in jax.devices():
            y = jax.device_put(np.ones((4, 4), np.float32), d)
            jnp.sum(y).block_until_ready()
    except Exception:
        pass


def kernel(x, weight):
    x = np.ascontiguousarray(np.asarray(x, dtype=np.float32).astype(np.float16))
    # frozen depthwise moving-average kernel: every tap is 1/W
    scale = float(np.asarray(weight).reshape(-1)[0])
    nc = build_nc(scale)
    shards = x.reshape(NCORES, ROWS, L)
    in_maps = [{"x": shards[c]} for c in range(NCORES)]
    _probe_devices()
    out = None
    for attempt in range(3):
        try:
            out = run_bass_kernel_spmd(nc, in_maps, core_ids=list(range(NCORES)))
            break
        except Exception:
            if attempt == 2:
                raise
            # a dirty previous client session can leave the device mesh
            # "unrecoverable"; a fresh PJRT client + probe clears it
            try:
                import jax

                jax.clear_backends()
            except Exception:
                pass
            _probe_devices()
    trend = np.concatenate(
        [np.asarray(out.results[c]["trend"], dtype=np.float32)[None] for c in range(NCORES)],
        axis=0,
    ).reshape(B, C, L)
    remainder = np.concatenate(
        [np.asarray(out.results[c]["remainder"], dtype=np.float32)[None] for c in range(NCORES)],
        axis=0,
    ).reshape(B, C, L)
    return trend, remainder


# revision 6
# speedup vs baseline: 1.4924x; 1.1267x over previous
"""Series decomposition: depthwise moving-average (box filter, W=25, replicate
padding) + remainder, data-parallel over batch across 8 NeuronCores.

Per core: x shard [4, 512, 4096] viewed as [2048, 4096] rows. For each
[128, 4096] row-tile, build a replicate-padded tile XP[128, 13+L+12], then
compute the sliding-window sum with a single DVE scan using the recurrence

    s[i] = s[i-1] + xp[i+12] - xp[i-13]

(tensor_tensor_scan: state = (data0 + state) - data1; the scan state is fp32
regardless of operand dtype), scale by the filter weight (1/25) on the scalar
engine, and subtract from x for the remainder. This is O(1) work per element
instead of O(W), so the kernel is HBM-bound.

HBM/DMA optimizations (the kernel sits at the HBM roofline):
- All device I/O is float16 (half-ulp 2^-11 ~ 5e-4 relative, far inside the
  2e-2 tolerance): halves HBM traffic vs fp32. fp32<->f16 conversion is
  host-side.
- The three DMA streams are spread across all three DMA-capable engines so
  their rings run concurrently: input on the SP HWDGE ring, trend on the
  Activation HWDGE ring, remainder on the GpSimd SWDGE path. A single ring
  serializes the per-DMA completion latency and measures ~190us/sweep; the
  3-ring spread measures ~145us, within ~5% of a pure-DMA floor probe.
- The scan stages its f16 window sums in the remainder tile (overwritten by
  the actual remainder afterwards), so no fp32 sum buffer is needed.
"""

import numpy as np

import concourse.bacc as bacc
import concourse.bass as bass
import concourse.mybir as mybir
from concourse.bass_utils import run_bass_kernel_spmd
from concourse.tile import TileContext

B, C, L, W = 32, 512, 4096, 25
PAD = W // 2  # 12
NCORES = 8
ROWS = (B // NCORES) * C  # 2048 rows per core
P = 128
NTILES = ROWS // P  # 16
LPAD = PAD + 1  # 13 left-pad cols (extra col feeds the scan's subtract lag)
XCOLS = LPAD + L + PAD  # 4121
BUFS = 6

FP32 = mybir.dt.float32
F16 = mybir.dt.float16


def build_nc(scale: float, rows: int = ROWS, l: int = L, repeats: int = 1,
             bufs: int = BUFS, swpipe: bool = True) -> bass.Bass:
    """repeats>1 re-runs the whole sweep inside one NEFF (timing harnesses
    use this to make device time dominate per-call dispatch overhead).

    swpipe defers each tile's remainder sub + DMA by one tile: in program
    order the DVE would otherwise sit idle between scan_i and sub_i waiting
    for the Act engine's mul_i (which only starts when scan_i ends); with the
    deferral the DVE runs scan_{i+1} during mul_i and sub_i's input is long
    ready when it issues."""
    ntiles = rows // P
    xcols = LPAD + l + PAD
    nc = bacc.Bacc(trn_type="TRN2")
    x = nc.dram_tensor("x", [rows, l], F16, kind="ExternalInput")
    trend = nc.dram_tensor("trend", [rows, l], F16, kind="ExternalOutput")
    remainder = nc.dram_tensor("remainder", [rows, l], F16, kind="ExternalOutput")

    with TileContext(nc) as tc:
        with tc.tile_pool(name="pool", bufs=bufs) as pool:
            pending = None  # (xp, t, r, rsl) of the previous tile

            def flush(pend):
                xp, t, r, rsl = pend
                nc.vector.tensor_sub(
                    out=r[:, :], in0=xp[:, LPAD : LPAD + l], in1=t[:, :]
                )
                nc.gpsimd.dma_start(out=remainder[rsl, :], in_=r[:, :])

            for i in range(ntiles * repeats):
                i = i % ntiles
                rsl = slice(i * P, (i + 1) * P)
                xp = pool.tile([P, xcols], F16, tag="xp")
                nc.sync.dma_start(out=xp[:, LPAD : LPAD + l], in_=x[rsl, :])
                # replicate ('edge') padding on both sides
                nc.vector.tensor_copy(
                    out=xp[:, 0:LPAD],
                    in_=xp[:, LPAD : LPAD + 1].to_broadcast((P, LPAD)),
                )
                nc.vector.tensor_copy(
                    out=xp[:, LPAD + l : xcols],
                    in_=xp[:, LPAD + l - 1 : LPAD + l].to_broadcast((P, PAD)),
                )
                # window sum at i=-1 plus the lagged element the first scan
                # step subtracts: sum of xp cols [-13..11] = XP[:, 0:25]
                init = pool.tile([P, 1], FP32, tag="init")
                nc.vector.tensor_reduce(
                    out=init[:, 0:1],
                    in_=xp[:, 0:W],
                    axis=mybir.AxisListType.X,
                    op=mybir.AluOpType.add,
                )
                # f16 window sums, staged in the remainder tile
                r = pool.tile([P, l], F16, tag="r")
                nc.vector.tensor_tensor_scan(
                    out=r[:, :],
                    data0=xp[:, W:xcols],
                    data1=xp[:, 0:l],
                    initial=init[:, 0:1],
                    op0=mybir.AluOpType.add,
                    op1=mybir.AluOpType.subtract,
                )
                t = pool.tile([P, l], F16, tag="t")
                nc.scalar.mul(t[:, :], r[:, :], scale)
                nc.scalar.dma_start(out=trend[rsl, :], in_=t[:, :])
                if swpipe:
                    if pending is not None:
                        flush(pending)
                    pending = (xp, t, r, rsl)
                else:
                    flush((xp, t, r, rsl))
            if pending is not None:
                flush(pending)
    nc.finalize()
    return nc


def _probe_devices():
    """Touch every NeuronCore with a trivial computation. After a previous
    client exits with in-flight bass executions, the first bass exec from a
    fresh client can fail with NRT_EXEC_UNIT_UNRECOVERABLE; a plain jax
    computation resets the state."""
    try:
        import jax
        import jax.numpy as jnp

        for d in jax.devices():
            y = jax.device_put(np.ones((4, 4), np.float32), d)
            jnp.sum(y).block_until_ready()
    except Exception:
        pass


def kernel(x, weight):
    x = np.ascontiguousarray(np.asarray(x, dtype=np.float32).astype(np.float16))
    # frozen depthwise moving-average kernel: every tap is 1/W
    scale = float(np.asarray(weight).reshape(-1)[0])
    nc = build_nc(scale)
    shards = x.reshape(NCORES, ROWS, L)
    in_maps = [{"x": shards[c]} for c in range(NCORES)]
    _probe_devices()
    out = None
    for attempt in range(3):
        try:
            out = run_bass_kernel_spmd(nc, in_maps, core_ids=list(range(NCORES)))
            break
        except Exception:
            if attempt == 2:
                raise
            # a dirty previous client session can leave the device mesh
            # "unrecoverable"; a fresh PJRT client + probe clears it
            try:
                import jax

                jax.clear_backends()
            except Exception:
                pass
            _probe_devices()
    trend = np.concatenate(
        [np.asarray(out.results[c]["trend"], dtype=np.float32)[None] for c in range(NCORES)],
        axis=0,
    ).reshape(B, C, L)
    remainder = np.concatenate(
        [np.asarray(out.results[c]["remainder"], dtype=np.float32)[None] for c in range(NCORES)],
        axis=0,
    ).reshape(B, C, L)
    return trend, remainder
